# revision 1
# baseline (speedup 1.0000x reference)
"""Multi-head attention (B=4, S=1024, H=1024, 16 heads) on 8 TRN2 NeuronCores.

Sharding: core c = (batch b = c//2, head-group g = c%2). Each core computes
attention for its batch over 8 of the 16 heads (a 512-wide column slice of
the QKV projections) plus the matching row-slice of the output projection.
The two partial output projections per batch are summed on the host
(row-parallel tensor-parallel unshard), where the bias bo is also added.

On-core dataflow (matmuls in f32r except the attention-weight matmul in
bf16; psum accumulation is fp32 throughout):
  QT[hd,s] = Wq_g^T x^T (+bq)   KT likewise (+bk)    V[t,hd] = x Wv_g (+bv)
  logitsT[t,s] per head: contraction over d=64; two heads packed in the PE
                         via tile_position row groups
  expT = exp(logitsT/8 + mask*NEG_INF)   (mask enters as per-partition bias)
  AVT[d,s] & colsum = [V_h | 1]^T @ expT (ones column makes psum row 64 the
                                          softmax denominator)
  attnT = AVT * (1/colsum broadcast)     (broadcast via k=1 ones matmul)
  out[s,n] += attnT-chunk^T @ Wo_g       (partial; host sums core pairs)
"""
import sys

sys.path.insert(0, "/opt/trn_rl_repo")

import ml_dtypes
import numpy as np

import concourse.bass as bass
import concourse.mybir as mybir
import concourse.tile as tile
from concourse import bacc
from concourse.bass_utils import run_bass_kernel_spmd

F32 = mybir.dt.float32
F32R = mybir.dt.float32r
BF16 = mybir.dt.bfloat16
EXPTYPE = BF16

B, S, H = 4, 1024, 1024
NH, HD = 16, 64
HPG = 8            # heads per group (per core)
GW = HPG * HD      # 512: group width
NEG_INF = -2.0 ** 32
NCORES = 8
HC = H // 128      # 8 contraction chunks over hidden
TC = S // 128      # 8 chunks over key positions t
SB = S // 512      # 2 halves of the s (query) axis

Exp = mybir.ActivationFunctionType.Exp


def _build(nrep=1):
    nc = bacc.Bacc("TRN2", target_bir_lowering=False, debug=False)

    xT = nc.dram_tensor("xT", [H, S], F32R, kind="ExternalInput")
    wq = nc.dram_tensor("wq", [H, GW], F32R, kind="ExternalInput")
    wk = nc.dram_tensor("wk", [H, GW], F32R, kind="ExternalInput")
    wv = nc.dram_tensor("wv", [H, GW], F32R, kind="ExternalInput")
    wo = nc.dram_tensor("wo", [GW, S], F32R, kind="ExternalInput")
    mask1 = nc.dram_tensor("mask1", [S], F32, kind="ExternalInput")
    bq1 = nc.dram_tensor("bq1", [GW], F32, kind="ExternalInput")
    bk1 = nc.dram_tensor("bk1", [GW], F32, kind="ExternalInput")
    bv1 = nc.dram_tensor("bv1", [1, GW], F32, kind="ExternalInput")
    ones = nc.dram_tensor("ones", [128, TC, HPG, 1], EXPTYPE, kind="ExternalInput")
    onecol = nc.dram_tensor("onecol", [1, HD], F32R, kind="ExternalInput")
    out = nc.dram_tensor("out", [S, H], F32, kind="ExternalOutput")

    with tile.TileContext(nc, pool_alloc_mode="stack") as tc:
      for _rep in range(nrep):
          # Pool releases must be LIFO, so the three big input pools
          # (xT/wqk/wv) are created LAST: they release mid-kernel (stack
          # rewinds) and p_wo then reuses their space.
          misc_cm = tc.tile_pool(name="misc", bufs=1); misc = misc_cm.__enter__()
          qkt_cm = tc.tile_pool(name="p_qkt", bufs=1); p_qkt = qkt_cm.__enter__()
          v_cm = tc.tile_pool(name="p_v", bufs=1); p_v = v_cm.__enter__()
          exp_cm = tc.tile_pool(name="p_exp", bufs=3); p_exp = exp_cm.__enter__()
          attn_cm = tc.tile_pool(name="p_attn", bufs=1)
          p_attn = attn_cm.__enter__()
          nrm_cm = tc.tile_pool(name="p_nrm", bufs=2); p_nrm = nrm_cm.__enter__()
          o_cm = tc.tile_pool(name="p_o", bufs=3); p_o = o_cm.__enter__()
          xT_cm = tc.tile_pool(name="p_xT", bufs=1); p_xT = xT_cm.__enter__()
          wqk_cm = tc.tile_pool(name="p_wqk", bufs=1); p_wqk = wqk_cm.__enter__()
          wv_cm = tc.tile_pool(name="p_wv", bufs=1); p_wv = wv_cm.__enter__()
          late = {"p_nrm": p_nrm}
          qkvps_cm = tc.tile_pool(name="ps_qkv", bufs=2, space="PSUM")
          ps_qkv = qkvps_cm.__enter__()
          lgps_cm = tc.tile_pool(name="ps_lg", bufs=2, space="PSUM")
          ps_lg = lgps_cm.__enter__()
          avps_cm = tc.tile_pool(name="ps_av", bufs=2, space="PSUM")
          ps_av = avps_cm.__enter__()

          # ---- tiny const DMAs first (cheap; they gate exp and copies) ----
          maskb = misc.tile([128, TC], F32, tag="maskb")
          mraw = misc.tile([128, TC], F32, tag="mraw")
          nc.sync.dma_start(out=mraw, in_=mask1.ap().rearrange("(c p) -> p c", p=128))
          nc.vector.tensor_scalar_mul(maskb, mraw, NEG_INF)
          bq_sb = misc.tile([128, 4], F32, tag="bq")
          bk_sb = misc.tile([128, 4], F32, tag="bk")
          nc.sync.dma_start(out=bq_sb, in_=bq1.ap().rearrange("(c p) -> p c", p=128))
          nc.sync.dma_start(out=bk_sb, in_=bk1.ap().rearrange("(c p) -> p c", p=128))
          bv_bc = misc.tile([128, GW], F32, tag="bv")
          nc.sync.dma_start(out=bv_bc, in_=bv1[0:1, :].to_broadcast((128, GW)))
          onecol_sb = misc.tile([1, HD], F32R, tag="onecol")
          nc.sync.dma_start(out=onecol_sb, in_=onecol[:, :])

          # ---- big input loads first: first QT matmul needs xT + wq blk0,
          # so those DMAs get top scheduler priority; weights split per block.
          xT_sb = p_xT.tile([128, HC, S], F32R, tag="xT")
          wq_sb = p_wqk.tile([128, HC, GW], F32R, tag="wq")
          wk_sb = p_wqk.tile([128, HC, GW], F32R, tag="wk")
          wv_sb = p_wv.tile([128, HC, GW], F32R, tag="wv")
          wq_r = wq.ap().rearrange("(c p) m -> p c m", p=128)
          wk_r = wk.ap().rearrange("(c p) m -> p c m", p=128)
          # operands of the very first matmuls first: wq blk0, then xT sh0
          nc.sync.dma_start(out=wq_sb[:, :, 0:128], in_=wq_r[:, :, 0:128])
          for c in range(HC):
              nc.sync.dma_start(out=xT_sb[:, c, 0:512], in_=xT[c * 128:(c + 1) * 128, 0:512])
          nc.sync.dma_start(out=wk_sb[:, :, 0:128], in_=wk_r[:, :, 0:128])
          for c in range(HC):
              nc.sync.dma_start(out=xT_sb[:, c, 512:1024], in_=xT[c * 128:(c + 1) * 128, 512:1024])
          for blk in range(1, 4):
              bs = slice(blk * 128, (blk + 1) * 128)
              nc.sync.dma_start(out=wq_sb[:, :, bs], in_=wq_r[:, :, bs])
          for blk in range(1, 4):
              bs = slice(blk * 128, (blk + 1) * 128)
              nc.sync.dma_start(out=wk_sb[:, :, bs], in_=wk_r[:, :, bs])
          nc.sync.dma_start(out=wv_sb, in_=wv.ap().rearrange("(c p) m -> p c m", p=128))


          QT_sb = p_qkt.tile([128, 4, S], F32R, tag="QT")
          KT_sb = p_qkt.tile([128, 4, S], F32R, tag="KT")
          V_sb = p_v.tile([128, TC, HPG, HD + 1], EXPTYPE, tag="V")
          nc.sync.dma_start(out=V_sb[:, :, :, HD:HD + 1], in_=ones.ap())
          late["attnT"] = p_attn.tile([128, 4, S], F32R, tag="attnT", name="attnT")

          def proj_half(dst, blk, sh, w_sb, b_sb):
              """dst[:, blk, sh-half] (+bias) = block of Wg^T x^T."""
              ps = ps_qkv.tile([128, 512], F32, tag="mm512")
              for c in range(HC):
                  nc.tensor.matmul(
                      ps, w_sb[:, c, blk * 128:(blk + 1) * 128],
                      xT_sb[:, c, sh * 512:(sh + 1) * 512],
                      start=(c == 0), stop=(c == HC - 1))
              nc.vector.tensor_scalar_add(
                  dst[:, blk, sh * 512:(sh + 1) * 512], ps, b_sb[:, blk:blk + 1])

          def v_chunk(tcn):
              """V_sb[:, tcn, :, 0:64] (+bv) = rows 128*tcn.. of x Wv_g."""
              ps = ps_qkv.tile([128, 512], F32, tag="mm512")
              for c in range(HC):
                  nc.tensor.matmul(
                      ps, xT_sb[:, c, tcn * 128:(tcn + 1) * 128], wv_sb[:, c, :],
                      start=(c == 0), stop=(c == HC - 1))
              nc.vector.tensor_add(
                  V_sb[:, tcn, :, 0:HD],
                  ps.rearrange("p (h d) -> p h d", h=HPG),
                  bv_bc.rearrange("p (h d) -> p h d", h=HPG))

          def logits_exp(pair, tcn, exp_dsts, split=False):
              """Packed pair of d=64 logit matmuls + exp for chunk tcn.
              split=True: one exp per s-half, so the first exps don't wait
              for the late-arriving second half of xT."""
              for i, (off, tp) in enumerate(((0, (0, 0)), (64, (64, 0)))):
                  lg = ps_lg.tile([128, 1024], F32, tag="lg")
                  for sh in range(SB):
                      nc.tensor.matmul(
                          lg[:, sh * 512:(sh + 1) * 512],
                          KT_sb[off:off + 64, pair, tcn * 128:(tcn + 1) * 128],
                          QT_sb[off:off + 64, pair, sh * 512:(sh + 1) * 512],
                          start=True, stop=True, tile_position=tp)
                      if split:
                          nc.scalar.activation(
                              out=exp_dsts[i][:, tcn, sh * 512:(sh + 1) * 512],
                              in_=lg[:, sh * 512:(sh + 1) * 512], func=Exp,
                              bias=maskb[:, tcn:tcn + 1], scale=0.125)
                  if not split:
                      nc.scalar.activation(
                          out=exp_dsts[i][:, tcn, :], in_=lg, func=Exp,
                          bias=maskb[:, tcn:tcn + 1], scale=0.125)

          def av_head_half(h, expT_h, sh, tail=False):
              """attnT rows for head h, s-half sh = normalized V_h^T @ expT_h.
              tail=True: route the bcast copy to ScalarE (idle after the last
              exp) so the DVE chain doesn't pace the tail."""
              off = (h % 2) * 64
              if True:
                  pav = ps_av.tile([HD + 1, 512], F32, tag="av",
                                    name=f"pav{h}_{sh}")
                  for tcn in range(TC):
                      nc.tensor.matmul(
                          pav, V_sb[:, tcn, h, :],
                          expT_h[:, tcn, sh * 512:(sh + 1) * 512],
                          start=(tcn == 0), stop=(tcn == TC - 1))
                  recip = late["p_nrm"].tile([1, 512], F32R, tag="recip")
                  with nc.allow_low_precision(reason="softmax denom recip to f32r"):
                      nc.vector.reciprocal(recip, pav[HD:HD + 1, :])
                  bps = ps_qkv.tile([HD, 512], F32, tag="mm512",
                                    name=f"bps{h}_{sh}")
                  nc.tensor.matmul(bps, onecol_sb, recip, start=True, stop=True)
                  bcast = late["p_nrm"].tile([HD, 512], F32, tag="bcast")
                  if tail:
                      nc.scalar.copy(bcast, bps)
                  else:
                      nc.vector.tensor_copy(bcast, bps)
                  nc.vector.tensor_mul(
                      late["attnT"][off:off + HD, h // 2, sh * 512:(sh + 1) * 512],
                      pav[0:HD, :], bcast)

          # ---------------- emission ----------------
          for sh in range(SB):
              proj_half(QT_sb, 0, sh, wq_sb, bq_sb)
          for sh in range(SB):
              proj_half(KT_sb, 0, sh, wk_sb, bk_sb)

          expT = {}
          for pair in range(4):
              ha, hb = 2 * pair, 2 * pair + 1
              if pair >= 1:
                  # free head 2p-2's expT slot ASAP: its exps finished during
                  # the previous loop, and the next pair's second tile waits
                  # on this slot
                  av_head_half(2 * pair - 2, expT[2 * pair - 2], 0)
              expT[ha] = p_exp.tile([128, TC, S], EXPTYPE, tag="expT", name=f"expT{ha}")
              expT[hb] = p_exp.tile([128, TC, S], EXPTYPE, tag="expT", name=f"expT{hb}")
              for tcn in range(TC):
                  # emit independent PE filler BEFORE logits: the logits
                  # matmul waits on its psum slot (paced by ACT exp), and the
                  # PE executes in order, so filler placed after it would
                  # head-of-line block
                  if pair >= 1 and tcn == 1:
                      av_head_half(2 * pair - 2, expT[2 * pair - 2], 1)
                  if pair >= 1 and tcn in (2, 4):
                      av_head_half(2 * pair - 1, expT[2 * pair - 1], tcn // 2 - 1)
                  if pair == 0:
                      if tcn > 0:
                          v_chunk(tcn)
                      if tcn in (1, 3):
                          proj_half(QT_sb, 1, tcn // 2, wq_sb, bq_sb)
                      elif tcn in (5, 7):
                          proj_half(KT_sb, 1, (tcn - 5) // 2, wk_sb, bk_sb)
                  elif pair < 3:
                      if tcn in (1, 3):
                          proj_half(QT_sb, pair + 1, tcn // 2, wq_sb, bq_sb)
                      elif tcn in (5, 7):
                          proj_half(KT_sb, pair + 1, (tcn - 5) // 2, wk_sb, bk_sb)
                  logits_exp(pair, tcn, (expT[ha], expT[hb]))
                  if pair == 0 and tcn == 0:
                      v_chunk(0)

          wv_cm.__exit__(None, None, None)
          wqk_cm.__exit__(None, None, None)
          xT_cm.__exit__(None, None, None)

          wo_cm = tc.tile_pool(name="p_wo", bufs=1)
          p_wo = wo_cm.__enter__()
          wo_sb = p_wo.tile([128, 4, S], F32R, tag="wo")
          nc.sync.dma_start(out=wo_sb, in_=wo.ap().rearrange("(c p) n -> p c n", p=128))

          for h in (6, 7):
              for sh in range(SB):
                  av_head_half(h, expT[h], sh, tail=True)

          attnT = late["attnT"]
          for st in range(TC):
              po = ps_lg.tile([128, 1024], F32, tag="lg", name=f"po{st}")
              for nh in range(SB):
                  for blk in range(4):
                      nc.tensor.matmul(
                          po[:, nh * 512:(nh + 1) * 512],
                          attnT[:, blk, st * 128:(st + 1) * 128],
                          wo_sb[:, blk, nh * 512:(nh + 1) * 512],
                          start=(blk == 0), stop=(blk == 3))
              o_sb = p_o.tile([128, 1024], F32, tag="o")
              if st == TC - 1:
                  # pipeline the last tile's copy->DMA in halves so the exit
                  # drain isn't gated by the full serial chain
                  for nh in range(SB):
                      cs = slice(nh * 512, (nh + 1) * 512)
                      nc.scalar.copy(o_sb[:, cs], po[:, cs])
                      nc.sync.dma_start(
                          out=out[st * 128:(st + 1) * 128, cs], in_=o_sb[:, cs])
              else:
                  nc.scalar.copy(o_sb, po)
                  nc.sync.dma_start(out=out[st * 128:(st + 1) * 128, :], in_=o_sb)

          for cm in (wo_cm, o_cm, nrm_cm, attn_cm, exp_cm, v_cm, qkt_cm,
                     misc_cm, avps_cm, lgps_cm, qkvps_cm):
              cm.__exit__(None, None, None)

    nc.compile()
    return nc


_NC = {}


def _get_nc(nrep=1):
    if nrep not in _NC:
        _NC[nrep] = _build(nrep)
    return _NC[nrep]


def kernel(x, mask, Wq, bq, Wk, bk, Wv, bv, Wo, bo, _trace=False):
    x = np.asarray(x, dtype=np.float32)
    mask = np.asarray(mask, dtype=np.float32)
    Wq, Wk, Wv, Wo = (np.asarray(w, dtype=np.float32) for w in (Wq, Wk, Wv, Wo))
    bq, bk, bv, bo = (np.asarray(b_, dtype=np.float32) for b_ in (bq, bk, bv, bo))

    nc = _get_nc()
    ones = np.ones((128, TC, HPG, 1), dtype=ml_dtypes.bfloat16)
    in_maps = []
    for c in range(NCORES):
        b, g = c // 2, c % 2
        sl = slice(g * GW, (g + 1) * GW)
        in_maps.append({
            "xT": np.ascontiguousarray(x[b].T),
            "wq": np.ascontiguousarray(Wq[:, sl]),
            "wk": np.ascontiguousarray(Wk[:, sl]),
            "wv": np.ascontiguousarray(Wv[:, sl]),
            "wo": np.ascontiguousarray(Wo[sl, :]),
            "mask1": np.ascontiguousarray(mask[b, 0, 0, :]),
            "bq1": np.ascontiguousarray(bq[sl]),
            "bk1": np.ascontiguousarray(bk[sl]),
            "bv1": np.ascontiguousarray(bv[sl]).reshape(1, GW),
            "ones": ones,
            "onecol": np.ones((1, HD), np.float32),
        })
    # First execution after NEFF load can race engine table initialization
    # (observed: garbage exp output on run 1 only). Warm up, then run.
    run_bass_kernel_spmd(nc, in_maps, core_ids=list(range(NCORES)))
    res = run_bass_kernel_spmd(
        nc, in_maps, core_ids=list(range(NCORES)), trace=_trace)
    kernel.last_results = res
    parts = [res.results[c]["out"] for c in range(NCORES)]
    return np.stack(
        [parts[2 * b] + parts[2 * b + 1] + bo for b in range(B)]
    ).astype(np.float32)



# revision 26
# speedup vs baseline: 1.3871x; 1.3871x over previous
"""Multi-head attention (B=4, S=1024, H=1024, 16 heads) on 8 TRN2 NeuronCores.

Sharding: core c = (batch b = c//2, head-group g = c%2). Each core computes
attention for its batch over 8 of the 16 heads (512-wide column slice of the
QKV projections, row slice of Wo). Host sums the two partial output
projections per batch and adds bo.

Per-core dataflow:
  QKV projections as fp8-e4m3 DoubleRow matmuls with 3-term hi/lo error
  compensation (x = xh+xl, W = Wh+Wl host-quantized at pow2 scales sx=16,
  sw=512; descale 2^-13 fused into the psum->SBUF bias pass); product =
  xh*Wh + xl*Wh + xh*Wl.  DoubleRow contracts two 128-chunks per pass at
  0.5 cycles/row -> 0.75x the bf16 matmul cost.
  logitsT[t,s] per head: bf16 Q,K, d=64 contraction, two heads packed in
  the PE via tile_position row groups.
  expT = exp(logitsT/8 + mask*NEG_INF) on ACT -- the single-engine
  bottleneck (64 instructions, one Exp table, nothing else runs on ACT).
  ACT paces the whole head loop, so PE filler work (remaining
  projections, V, AV, transposes) is spread one ~0.6us slice per exp
  chunk with explicit deadlines.
  AV reoriented: expT chunk is the STATIONARY operand [128t x 128s], V
  (with a ones column: softmax denominator) is the moving operand
  [128t x 65] -> out[s-part, d|denom] at 65 rows/pass, half the cost of
  the V-stationary orientation; the denominator lands per-partition so
  normalization is one DVE tensor op (no broadcast matmul).
  attn[s,(h d)] -> attnT[(h d),s] via PE transpose against identity
  (psum slot borrowed from the AV pool via bitcast), then
  out[s,n] = attnT^T @ Wo in bf16, DVE psum->SBUF copy, DMA out.
"""
import sys

sys.path.insert(0, "/opt/trn_rl_repo")

import ml_dtypes
import numpy as np

import concourse.bass as bass
import concourse.mybir as mybir
import concourse.tile as tile
from concourse import bacc
from concourse.bass_utils import run_bass_kernel_spmd

F32 = mybir.dt.float32
BF16 = mybir.dt.bfloat16
E4 = mybir.dt.float8e4
DR = mybir.MatmulPerfMode.DoubleRow
Exp = mybir.ActivationFunctionType.Exp
MULT = mybir.AluOpType.mult
ADD = mybir.AluOpType.add

B, S, H = 4, 1024, 1024
NH, HD = 16, 64
HPG = 8            # heads per group (per core)
GW = HPG * HD      # 512
NEG_INF = -2.0 ** 32
NCORES = 8
HC = H // 128      # 8 contraction chunks over hidden
TC = S // 128      # 8 chunks over key positions t
SX = 16.0          # x fp8 scale
SW = 512.0         # W fp8 scale
DESCALE = 1.0 / (SX * SW)   # 2^-13


def _build(nrep=1):
    nc = bacc.Bacc("TRN2", target_bir_lowering=False, debug=False)

    # weight layouts are blk-major so per-blk DMA slices are contiguous
    # >=512B runs (sub-512B DMA elements pay 2x latency)
    xh = nc.dram_tensor("xh", [128, HC, S], E4, kind="ExternalInput")
    xl = nc.dram_tensor("xl", [128, HC, S], E4, kind="ExternalInput")
    wqh = nc.dram_tensor("wqh", [128, 4, HC, 128], E4, kind="ExternalInput")
    wql = nc.dram_tensor("wql", [128, 4, HC, 128], E4, kind="ExternalInput")
    wkh = nc.dram_tensor("wkh", [128, 4, HC, 128], E4, kind="ExternalInput")
    wkl = nc.dram_tensor("wkl", [128, 4, HC, 128], E4, kind="ExternalInput")
    wvh = nc.dram_tensor("wvh", [128, 2, HC, 256], E4, kind="ExternalInput")
    wvl = nc.dram_tensor("wvl", [128, 2, HC, 256], E4, kind="ExternalInput")
    wo = nc.dram_tensor("wo", [128, 4, S], BF16, kind="ExternalInput")
    mask1 = nc.dram_tensor("mask1", [S], F32, kind="ExternalInput")
    bq1 = nc.dram_tensor("bq1", [GW], F32, kind="ExternalInput")
    bk1 = nc.dram_tensor("bk1", [GW], F32, kind="ExternalInput")
    bv1 = nc.dram_tensor("bv1", [1, GW], F32, kind="ExternalInput")
    ones = nc.dram_tensor("ones", [128, TC, HPG, 1], BF16, kind="ExternalInput")
    ident = nc.dram_tensor("ident", [128, 128], BF16, kind="ExternalInput")
    out = nc.dram_tensor("out", [S, H], BF16, kind="ExternalOutput")

    with tile.TileContext(nc, pool_alloc_mode="stack") as tc:
      for _rep in range(nrep):
        misc_cm = tc.tile_pool(name="misc", bufs=1); misc = misc_cm.__enter__()
        x_cm = tc.tile_pool(name="p_x", bufs=1); p_x = x_cm.__enter__()
        w_cm = tc.tile_pool(name="p_w", bufs=1); p_w = w_cm.__enter__()
        qkt_cm = tc.tile_pool(name="p_qkt", bufs=1); p_qkt = qkt_cm.__enter__()
        v_cm = tc.tile_pool(name="p_v", bufs=1); p_v = v_cm.__enter__()
        exp_cm = tc.tile_pool(name="p_exp", bufs=3); p_exp = exp_cm.__enter__()
        attn_cm = tc.tile_pool(name="p_attn", bufs=1); p_attn = attn_cm.__enter__()
        r4_cm = tc.tile_pool(name="p_r4", bufs=2); p_r4 = r4_cm.__enter__()
        at_cm = tc.tile_pool(name="p_at", bufs=1); p_at = at_cm.__enter__()
        o_cm = tc.tile_pool(name="p_o", bufs=4); p_o = o_cm.__enter__()
        # PSUM: proj(2 banks) + po(2) + lg(4) = 8. lg releases after the
        # head loop; the output-projection pool takes its 4 banks. po stays
        # for the tail transposes (borrowed slots).
        prps_cm = tc.tile_pool(name="ps_proj", bufs=2, space="PSUM")
        ps_proj = prps_cm.__enter__()
        pops_cm = tc.tile_pool(name="ps_po", bufs=2, space="PSUM")
        ps_po = pops_cm.__enter__()
        lgps_cm = tc.tile_pool(name="ps_lg", bufs=2, space="PSUM")
        ps_lg = lgps_cm.__enter__()

        # ---- DMAs: first-needed first; tiny consts slot in before their
        # first consumer. ----
        xh_sb = p_x.tile([128, HC, S], E4, tag="xh")
        xl_sb = p_x.tile([128, HC, S], E4, tag="xl")
        wqh_sb = p_w.tile([128, 4, HC, 128], E4, tag="wqh")
        wql_sb = p_w.tile([128, 4, HC, 128], E4, tag="wql")
        wkh_sb = p_w.tile([128, 4, HC, 128], E4, tag="wkh")
        wkl_sb = p_w.tile([128, 4, HC, 128], E4, tag="wkl")
        wvh_sb = p_w.tile([128, 2, HC, 256], E4, tag="wvh")
        wvl_sb = p_w.tile([128, 2, HC, 256], E4, tag="wvl")
        wo_sb = p_w.tile([128, 4, S], BF16, tag="wo")
        bq_sb = misc.tile([128, 4], F32, tag="bq")
        bk_sb = misc.tile([128, 4], F32, tag="bk")
        maskb = misc.tile([128, TC], F32, tag="maskb")
        mraw = misc.tile([128, TC], F32, tag="mraw")
        bv_bc = misc.tile([128, GW], F32, tag="bv")
        id_sb = misc.tile([128, 128], BF16, tag="ident")

        def dma(dst, src):
            nc.sync.dma_start(out=dst, in_=src)

        # startup chain for the first exp: QT-mb0 (xh+xl, wq blk0) then
        # KT-mb0; everything else ordered by first use (DMA is a single
        # serial queue in the cost model).
        dma(wqh_sb[:, 0], wqh[:, 0])
        dma(bq_sb, bq1.ap().rearrange("(c p) -> p c", p=128))
        dma(xh_sb[:, :, 0:512], xh[:, :, 0:512])
        dma(wql_sb[:, 0], wql[:, 0])
        dma(xl_sb[:, :, 0:512], xl[:, :, 0:512])
        dma(wkh_sb[:, 0], wkh[:, 0])
        dma(wkl_sb[:, 0], wkl[:, 0])
        dma(bk_sb, bk1.ap().rearrange("(c p) -> p c", p=128))
        dma(mraw, mask1.ap().rearrange("(c p) -> p c", p=128))
        dma(xh_sb[:, :, 512:1024], xh[:, :, 512:1024])
        dma(xl_sb[:, :, 512:1024], xl[:, :, 512:1024])
        # blk1 weights: QK-mb1 filler halves start at h0c4
        for w_sb, w_d in ((wqh_sb, wqh), (wql_sb, wql),
                          (wkh_sb, wkh), (wkl_sb, wkl)):
            dma(w_sb[:, 1], w_d[:, 1])
        # wv first half (heads 0-3): V fillers start at h1c4
        dma(wvh_sb[:, 0], wvh[:, 0])
        dma(wvl_sb[:, 0], wvl[:, 0])
        dma(bv_bc, bv1[0:1, :].to_broadcast((128, GW)))
        dma(id_sb, ident[:, :])
        QT = p_qkt.tile([128, 4, S], BF16, tag="QT")
        KT = p_qkt.tile([128, 4, S], BF16, tag="KT")
        V_sb = p_v.tile([128, TC, HPG, HD + 1], BF16, tag="V")
        dma(V_sb[:, :, :, HD:HD + 1], ones.ap())
        for blk in range(2, 4):
            for w_sb, w_d in ((wqh_sb, wqh), (wql_sb, wql),
                              (wkh_sb, wkh), (wkl_sb, wkl)):
                dma(w_sb[:, blk], w_d[:, blk])
        # wv second half (heads 4-7): V-b fillers start at h4c3
        dma(wvh_sb[:, 1], wvh[:, 1])
        dma(wvl_sb[:, 1], wvl[:, 1])
        for blk in range(4):
            dma(wo_sb[:, blk, :], wo[:, blk, :])
        attn_sb = p_attn.tile([128, TC, GW], BF16, tag="attn")
        attnT = p_at.tile([128, 4, S], BF16, tag="attnT")

        # ---- compute emitters ----
        pq_state = {}

        def proj_qk_half(dst, b_sb, wh_t, wl_t, mb, nh, nb):
            """Half (256 s-cols) of a 3-term fp8-DR projection block.
            nb=0 allocates the [128,512] psum tile; nb=1 emits the fused
            descale+bias DVE pass over the full 512."""
            key = (id(dst), mb, nh)
            if nb == 0:
                pq_state[key] = ps_proj.tile(
                    [128, 512], F32, tag="pproj", name=f"pq{mb}_{nh}")
            ps = pq_state[key]
            n0 = nh * 512
            ns = slice(n0 + nb * 256, n0 + (nb + 1) * 256)
            os_ = slice(nb * 256, (nb + 1) * 256)
            first = True
            for cp in range(4):
                cs = slice(2 * cp, 2 * cp + 2)
                for wt, xt in ((wh_t, xh_sb), (wl_t, xh_sb), (wh_t, xl_sb)):
                    last = (cp == 3 and xt is xl_sb)
                    nc.tensor.matmul(
                        ps[:, os_], wt[:, mb, cs, :], xt[:, cs, ns],
                        start=first, stop=last, perf_mode=DR,
                        skip_group_check=True)
                    first = False
            if nb == 1:
                nc.vector.tensor_scalar(
                    dst[:, mb, n0:n0 + 512], ps, DESCALE,
                    b_sb[:, mb:mb + 1], MULT, ADD)
                del pq_state[key]

        def proj_v_half(tb, nb):
            """V rows 128*tb for head-group nb (4 heads, 256 wv-cols);
            self-contained: 12 DR matmuls + fused descale+bias pass."""
            ps = ps_proj.tile([128, 512], F32, tag="pproj",
                              name=f"pv{tb}_{nb}")
            ts = slice(tb * 128, (tb + 1) * 128)
            first = True
            for cp in range(4):
                cs = slice(2 * cp, 2 * cp + 2)
                for wt, xt in ((wvh_sb, xh_sb), (wvl_sb, xh_sb),
                               (wvh_sb, xl_sb)):
                    last = (cp == 3 and xt is xl_sb)
                    nc.tensor.matmul(
                        ps[:, 0:256], xt[:, cs, ts], wt[:, nb, cs, :],
                        start=first, stop=last, perf_mode=DR,
                        skip_group_check=True)
                    first = False
            nc.vector.scalar_tensor_tensor(
                V_sb[:, tb, 4 * nb:4 * nb + 4, 0:HD],
                ps[:, 0:256].rearrange("p (h d) -> p h d", h=4), DESCALE,
                bv_bc[:, nb * 256:(nb + 1) * 256].rearrange(
                    "p (h d) -> p h d", h=4), MULT, ADD)

        def logits_exp(h, eT, tcn):
            mb, off = h // 2, (h % 2) * 64
            lg = ps_lg.tile([128, 1024], F32, tag="lg")
            for sh in range(2):
                nc.tensor.matmul(
                    lg[:, sh * 512:(sh + 1) * 512],
                    KT[off:off + 64, mb, tcn * 128:(tcn + 1) * 128],
                    QT[off:off + 64, mb, sh * 512:(sh + 1) * 512],
                    start=True, stop=True, tile_position=(off, 0))
            nc.scalar.activation(
                out=eT[:, tcn, :], in_=lg, func=Exp,
                bias=maskb[:, tcn:tcn + 1], scale=0.125)

        expT = {}
        av_state = {}

        def av_part(h, sbg, tcs):
            """Partial AV accumulation for head h, s-blocks 4*sbg.., over
            the t-chunks in `tcs` (split emission so av(7)'s last chunk is
            the only tail PE work)."""
            key = (h, sbg)
            if key not in av_state:
                av_state[key] = ps_po.tile(
                    [128, 4, HD + 1], F32, tag="po", name=f"po{h}_{sbg}")
            po = av_state[key]
            eT = expT[h]
            for i in range(4):
                sb = sbg * 4 + i
                for tcn in tcs:
                    nc.tensor.matmul(
                        po[:, i, :],
                        eT[:, tcn, sb * 128:(sb + 1) * 128],
                        V_sb[:, tcn, h, :],
                        start=(tcn == 0), stop=(tcn == TC - 1))

        def av_fin(h, sbg):
            po = av_state.pop((h, sbg))
            r4 = p_r4.tile([128, 4, 1], F32, tag="r4")
            nc.vector.reciprocal(r4, po[:, :, HD:HD + 1])
            nc.vector.tensor_mul(
                attn_sb[:, sbg * 4:(sbg + 1) * 4, h * HD:(h + 1) * HD],
                po[:, :, 0:HD], r4[:, :, 0:1].to_broadcast((128, 4, HD)))

        def av(h, sbg):
            av_part(h, sbg, range(TC))
            av_fin(h, sbg)

        def transpose_blk(blk):
            """attn s-block columns of gw-block blk -> attnT rows.
            Borrows proj-pool psum slots (bf16 view via bitcast)."""
            for sb in range(TC):
                bt = ps_proj.tile([128, 512], F32, tag="pproj",
                                  name=f"pt{blk}_{sb}")
                pt = bt[:, 0:128].bitcast(BF16)[:, 0:128]
                nc.tensor.matmul(
                    pt, attn_sb[:, sb, blk * 128:(blk + 1) * 128], id_sb,
                    start=True, stop=True, is_transpose=True)
                nc.vector.tensor_copy(
                    attnT[:, blk, sb * 128:(sb + 1) * 128], pt)

        # ---------------- emission ----------------
        # QT-nh0, KT-nh0 first (they only need the first x halves), then
        # the nh1 halves as x's second half lands.
        for nb in range(2):
            proj_qk_half(QT, bq_sb, wqh_sb, wql_sb, 0, 0, nb)
        for nb in range(2):
            proj_qk_half(KT, bk_sb, wkh_sb, wkl_sb, 0, 0, nb)
        for nb in range(2):
            proj_qk_half(QT, bq_sb, wqh_sb, wql_sb, 0, 1, nb)
        for nb in range(2):
            proj_qk_half(KT, bk_sb, wkh_sb, wkl_sb, 0, 1, nb)
        nc.vector.tensor_scalar_mul(maskb, mraw, NEG_INF)

        def F_qk(dst_b, mb, nh, nb):
            dst, b_ = (QT, bq_sb) if dst_b == "q" else (KT, bk_sb)
            wh_t, wl_t = ((wqh_sb, wql_sb) if dst_b == "q"
                          else (wkh_sb, wkl_sb))
            return lambda: proj_qk_half(dst, b_, wh_t, wl_t, mb, nh, nb)

        def F_v(tb, nb):
            return lambda: proj_v_half(tb, nb)

        def F_av(h, g):
            return lambda: av(h, g)

        def F_avp(h, g, tcs):
            return lambda: av_part(h, g, tcs)

        def F_tp(blk):
            return lambda: transpose_blk(blk)

        # filler schedule: [h][tcn] -> list of emitters. Budget ~0.6us of
        # PE filler per 1.04us exp chunk; no fillers on the first chunks
        # (their inputs aren't DMA'd yet and PE head-of-line blocking
        # would starve ACT). Deadlines: QT/KT-mb(k) before head 2k's
        # logits, V head-group a (0-3) before av(0) at h2c5, group b
        # before av(4) at h6c4, av(h) done before head h+3 starts (expT
        # pool bufs=3), transpose blk b after av(2b+1), av(7) split so
        # only its last t-chunk trails the final exp.
        FILL = {
            (0, 4): [F_qk("q", 1, 0, 0)],
            (0, 5): [F_qk("q", 1, 0, 1)],
            (0, 6): [F_qk("q", 1, 1, 0)],
            (0, 7): [F_qk("q", 1, 1, 1)],
            (1, 0): [F_qk("k", 1, 0, 0)],
            (1, 1): [F_qk("k", 1, 0, 1)],
            (1, 2): [F_qk("k", 1, 1, 0)],
            (1, 3): [F_qk("k", 1, 1, 1)],
            (1, 4): [F_v(0, 0)], (1, 5): [F_v(1, 0)],
            (1, 6): [F_v(2, 0)], (1, 7): [F_v(3, 0)],
            (2, 0): [F_v(4, 0)], (2, 1): [F_v(5, 0)],
            (2, 2): [F_v(6, 0)], (2, 3): [F_v(7, 0)],
            (2, 4): [F_qk("q", 2, 0, 0)],
            (2, 5): [F_av(0, 0)],
            (2, 6): [F_qk("q", 2, 0, 1)],
            (2, 7): [F_av(0, 1)],
            (3, 0): [F_qk("q", 2, 1, 0)],
            (3, 1): [F_av(1, 0)],
            (3, 2): [F_qk("q", 2, 1, 1)],
            (3, 3): [F_qk("k", 2, 0, 0)],
            (3, 4): [F_qk("k", 2, 0, 1)],
            (3, 5): [F_av(1, 1)],
            (3, 6): [F_qk("k", 2, 1, 0)],
            (3, 7): [F_qk("k", 2, 1, 1)],
            (4, 0): [F_qk("q", 3, 0, 0)],
            (4, 1): [F_av(2, 0)],
            (4, 2): [F_qk("q", 3, 0, 1)],
            (4, 3): [F_v(0, 1)],
            (4, 4): [F_qk("q", 3, 1, 0)],
            (4, 5): [F_av(2, 1)],
            (4, 6): [F_qk("q", 3, 1, 1)],
            (4, 7): [F_v(1, 1)],
            (5, 0): [F_qk("k", 3, 0, 0)],
            (5, 1): [F_av(3, 0)],
            (5, 2): [F_qk("k", 3, 0, 1)],
            (5, 3): [F_qk("k", 3, 1, 0)],
            (5, 4): [F_qk("k", 3, 1, 1)],
            (5, 5): [F_av(3, 1)],
            (5, 6): [F_v(2, 1)],
            (5, 7): [F_v(3, 1)],
            (6, 0): [F_v(4, 1)],
            (6, 1): [F_v(5, 1)],
            (6, 2): [F_v(6, 1)],
            (6, 3): [F_v(7, 1)],
            (6, 4): [F_av(4, 0)],
            (6, 5): [F_tp(1)],
            (6, 6): [F_av(4, 1)],
            (6, 7): [F_tp(0)],
            (7, 0): [F_av(5, 0)],
            (7, 1): [F_av(5, 1)],
            (7, 2): [F_av(6, 0)],
            (7, 3): [F_av(6, 1)],
            (7, 4): [F_tp(2)],
        }

        for h in range(HPG):
            expT[h] = p_exp.tile([128, TC, S], BF16, tag="expT",
                                 name=f"expT{h}")
            for tcn in range(TC):
                for f in FILL.get((h, tcn), ()):
                    f()
                logits_exp(h, expT[h], tcn)

        av(7, 0)
        av(7, 1)
        transpose_blk(3)

        lgps_cm.__exit__(None, None, None)
        opps_cm = tc.tile_pool(name="ps_op", bufs=2, space="PSUM")
        ps_op = opps_cm.__enter__()

        for st in range(TC):
            op = ps_op.tile([128, 1024], F32, tag="op")
            for nh in range(2):
                for blk in range(4):
                    nc.tensor.matmul(
                        op[:, nh * 512:(nh + 1) * 512],
                        attnT[:, blk, st * 128:(st + 1) * 128],
                        wo_sb[:, blk, nh * 512:(nh + 1) * 512],
                        start=(blk == 0), stop=(blk == 3))
            o_sb = p_o.tile([128, 1024], BF16, tag="o")
            for nh in range(2):
                cs = slice(nh * 512, (nh + 1) * 512)
                nc.vector.tensor_copy(o_sb[:, cs], op[:, cs])
                dma(out[st * 128:(st + 1) * 128, cs], o_sb[:, cs])

        for cm in (opps_cm, pops_cm, prps_cm, o_cm, at_cm, r4_cm, attn_cm,
                   exp_cm, v_cm, qkt_cm, w_cm, x_cm, misc_cm):
            cm.__exit__(None, None, None)

    nc.compile()
    return nc


_NC = {}


def _get_nc(nrep=1):
    if nrep not in _NC:
        _NC[nrep] = _build(nrep)
    return _NC[nrep]


E4NP = ml_dtypes.float8_e4m3


def _q8(a):
    """fp8 hi/lo split: a ~= hi + lo (both e4m3)."""
    hi = a.astype(E4NP)
    lo = (a - hi.astype(np.float32)).astype(E4NP)
    return hi, lo


def _chunk128(a):
    """[1024, M] -> [128, 8, M] partition-major chunking of the rows."""
    m = a.shape[1]
    return np.ascontiguousarray(a.reshape(HC, 128, m).transpose(1, 0, 2))


def kernel(x, mask, Wq, bq, Wk, bk, Wv, bv, Wo, bo, _trace=False):
    x = np.asarray(x, dtype=np.float32)
    mask = np.asarray(mask, dtype=np.float32)
    Wq, Wk, Wv, Wo = (np.asarray(w, dtype=np.float32) for w in (Wq, Wk, Wv, Wo))
    bq, bk, bv, bo = (np.asarray(b_, dtype=np.float32) for b_ in (bq, bk, bv, bo))

    nc = _get_nc()
    ones = np.ones((128, TC, HPG, 1), dtype=ml_dtypes.bfloat16)
    ident = np.eye(128, dtype=ml_dtypes.bfloat16)
    in_maps = []
    for c in range(NCORES):
        b, g = c // 2, c % 2
        sl = slice(g * GW, (g + 1) * GW)
        xh_, xl_ = _q8(np.ascontiguousarray(x[b].T) * SX)
        wq_h, wq_l = _q8(Wq[:, sl] * SW)
        wk_h, wk_l = _q8(Wk[:, sl] * SW)
        wv_h, wv_l = _q8(Wv[:, sl] * SW)

        def wblk(a, nblk):
            # [1024, 512] -> [128, nblk, HC, 512//nblk] (partition-major
            # rows, blk-major cols so per-blk DMA slices are contiguous)
            c = a.reshape(HC, 128, nblk, GW // nblk)
            return np.ascontiguousarray(c.transpose(1, 2, 0, 3))

        in_maps.append({
            "xh": _chunk128(xh_), "xl": _chunk128(xl_),
            "wqh": wblk(wq_h, 4), "wql": wblk(wq_l, 4),
            "wkh": wblk(wk_h, 4), "wkl": wblk(wk_l, 4),
            "wvh": wblk(wv_h, 2), "wvl": wblk(wv_l, 2),
            "wo": np.ascontiguousarray(
                Wo[sl, :].reshape(4, 128, S).transpose(1, 0, 2)
            ).astype(ml_dtypes.bfloat16),
            "mask1": np.ascontiguousarray(mask[b, 0, 0, :]),
            "bq1": np.ascontiguousarray(bq[sl]),
            "bk1": np.ascontiguousarray(bk[sl]),
            "bv1": np.ascontiguousarray(bv[sl]).reshape(1, GW),
            "ones": ones,
            "ident": ident,
        })
    # First execution after NEFF load can race engine table initialization.
    # Warm up, then run.
    run_bass_kernel_spmd(nc, in_maps, core_ids=list(range(NCORES)))
    res = run_bass_kernel_spmd(
        nc, in_maps, core_ids=list(range(NCORES)), trace=_trace)
    kernel.last_results = res
    parts = [res.results[c]["out"].astype(np.float32) for c in range(NCORES)]
    return np.stack(
        [parts[2 * b] + parts[2 * b + 1] + bo for b in range(B)]
    ).astype(np.float32)


# revision 54
# speedup vs baseline: 1.3924x; 1.0038x over previous
"""Multi-head attention (B=4, S=1024, H=1024, 16 heads) on 8 TRN2 NeuronCores.

Sharding: core c = (batch b = c//2, head-group g = c%2). Each core computes
attention for its batch over 8 of the 16 heads (512-wide column slice of the
QKV projections, row slice of Wo). Host sums the two partial output
projections per batch and adds bo.

Per-core dataflow:
  QKV projections as fp8-e4m3 DoubleRow matmuls with 3-term hi/lo error
  compensation (x = xh+xl, W = Wh+Wl host-quantized at pow2 scales sx=16,
  sw=512; descale 2^-13 fused into the psum->SBUF bias pass); product =
  xh*Wh + xl*Wh + xh*Wl.  DoubleRow contracts two 128-chunks per pass at
  0.5 cycles/row -> 0.75x the bf16 matmul cost.
  logitsT[t,s] per head: bf16 Q,K, d=64 contraction, two heads packed in
  the PE via tile_position row groups.
  expT = exp(logitsT/8 + mask*NEG_INF) on ACT -- the single-engine
  bottleneck (64 instructions, one Exp table, nothing else runs on ACT).
  ACT paces the whole head loop, so PE filler work (remaining
  projections, V, AV, transposes) is spread one ~0.6us slice per exp
  chunk with explicit deadlines.
  AV reoriented: expT chunk is the STATIONARY operand [128t x 128s], V
  (with a ones column: softmax denominator) is the moving operand
  [128t x 65] -> out[s-part, d|denom] at 65 rows/pass, half the cost of
  the V-stationary orientation; the denominator lands per-partition so
  normalization is one DVE tensor op (no broadcast matmul).
  attn[s,(h d)] -> attnT[(h d),s] via PE transpose against identity
  (psum slot borrowed from the AV pool via bitcast), then
  out[s,n] = attnT^T @ Wo in bf16, DVE psum->SBUF copy, DMA out.
"""
import sys

sys.path.insert(0, "/opt/trn_rl_repo")

import ml_dtypes
import numpy as np

import concourse.bass as bass
import concourse.mybir as mybir
import concourse.tile as tile
from concourse import bacc
from concourse.bass_utils import run_bass_kernel_spmd

F32 = mybir.dt.float32
BF16 = mybir.dt.bfloat16
E4 = mybir.dt.float8e4
DR = mybir.MatmulPerfMode.DoubleRow
Exp = mybir.ActivationFunctionType.Exp
MULT = mybir.AluOpType.mult
ADD = mybir.AluOpType.add

B, S, H = 4, 1024, 1024
NH, HD = 16, 64
HPG = 8            # heads per group (per core)
GW = HPG * HD      # 512
NEG_INF = -2.0 ** 32
NCORES = 8
HC = H // 128      # 8 contraction chunks over hidden
TC = S // 128      # 8 chunks over key positions t
SX = 16.0          # x fp8 scale
SW = 512.0         # W fp8 scale
DESCALE = 1.0 / (SX * SW)   # 2^-13


def _build(nrep=1):
    nc = bacc.Bacc("TRN2", target_bir_lowering=False, debug=False)

    # weight layouts are blk-major so per-blk DMA slices are contiguous
    # >=512B runs (sub-512B DMA elements pay 2x latency)
    xh = nc.dram_tensor("xh", [128, HC, S], E4, kind="ExternalInput")
    xl = nc.dram_tensor("xl", [128, HC, S], E4, kind="ExternalInput")
    wqh = nc.dram_tensor("wqh", [128, 4, HC, 128], E4, kind="ExternalInput")
    wql = nc.dram_tensor("wql", [128, 4, HC, 128], E4, kind="ExternalInput")
    wkh = nc.dram_tensor("wkh", [128, 4, HC, 128], E4, kind="ExternalInput")
    wkl = nc.dram_tensor("wkl", [128, 4, HC, 128], E4, kind="ExternalInput")
    wvh = nc.dram_tensor("wvh", [128, 2, HC, 256], E4, kind="ExternalInput")
    wvl = nc.dram_tensor("wvl", [128, 2, HC, 256], E4, kind="ExternalInput")
    woh = nc.dram_tensor("woh", [128, 4, S], E4, kind="ExternalInput")
    wol = nc.dram_tensor("wol", [128, 4, S], E4, kind="ExternalInput")
    # consts packs [bq | bk | mask] as [128, 4+4+8] (one DMA instead of 3)
    consts = nc.dram_tensor("consts", [128, 16], F32, kind="ExternalInput")
    bv1 = nc.dram_tensor("bv1", [1, GW], F32, kind="ExternalInput")
    ones = nc.dram_tensor("ones", [128, TC, HPG, 1], BF16, kind="ExternalInput")
    ident = nc.dram_tensor("ident", [128, 128], BF16, kind="ExternalInput")
    out = nc.dram_tensor("out", [S, H], BF16, kind="ExternalOutput")

    with tile.TileContext(nc, pool_alloc_mode="stack") as tc:
      for _rep in range(nrep):
        misc_cm = tc.tile_pool(name="misc", bufs=1); misc = misc_cm.__enter__()
        x_cm = tc.tile_pool(name="p_x", bufs=1); p_x = x_cm.__enter__()
        w_cm = tc.tile_pool(name="p_w", bufs=1); p_w = w_cm.__enter__()
        qkt_cm = tc.tile_pool(name="p_qkt", bufs=1); p_qkt = qkt_cm.__enter__()
        v_cm = tc.tile_pool(name="p_v", bufs=1); p_v = v_cm.__enter__()
        exp_cm = tc.tile_pool(name="p_exp", bufs=3); p_exp = exp_cm.__enter__()
        attn_cm = tc.tile_pool(name="p_attn", bufs=1); p_attn = attn_cm.__enter__()
        r4_cm = tc.tile_pool(name="p_r4", bufs=2); p_r4 = r4_cm.__enter__()
        at_cm = tc.tile_pool(name="p_at", bufs=1); p_at = at_cm.__enter__()
        o_cm = tc.tile_pool(name="p_o", bufs=4); p_o = o_cm.__enter__()
        # PSUM: proj(2 banks) + po(2) + lg(4) = 8. lg releases after the
        # head loop; the output-projection pool takes its 4 banks. po stays
        # for the tail transposes (borrowed slots).
        prps_cm = tc.tile_pool(name="ps_proj", bufs=2, space="PSUM")
        ps_proj = prps_cm.__enter__()
        pops_cm = tc.tile_pool(name="ps_po", bufs=2, space="PSUM")
        ps_po = pops_cm.__enter__()
        lgps_cm = tc.tile_pool(name="ps_lg", bufs=2, space="PSUM")
        ps_lg = lgps_cm.__enter__()

        # ---- DMAs: first-needed first; tiny consts slot in before their
        # first consumer. ----
        xh_sb = p_x.tile([128, HC, S], E4, tag="xh")
        xl_sb = p_x.tile([128, HC, S], E4, tag="xl")
        wqh_sb = p_w.tile([128, 4, HC, 128], E4, tag="wqh")
        wql_sb = p_w.tile([128, 4, HC, 128], E4, tag="wql")
        wkh_sb = p_w.tile([128, 4, HC, 128], E4, tag="wkh")
        wkl_sb = p_w.tile([128, 4, HC, 128], E4, tag="wkl")
        wvh_sb = p_w.tile([128, 2, HC, 256], E4, tag="wvh")
        wvl_sb = p_w.tile([128, 2, HC, 256], E4, tag="wvl")
        woh_sb = p_w.tile([128, 4, S], E4, tag="woh")
        wol_sb = p_w.tile([128, 4, S], E4, tag="wol")
        c_sb = misc.tile([128, 16], F32, tag="consts")
        bq_sb = c_sb[:, 0:4]
        bk_sb = c_sb[:, 4:8]
        maskb = misc.tile([128, TC], F32, tag="maskb")
        bv_bc = misc.tile([128, GW], F32, tag="bv")
        id_sb = misc.tile([128, 128], BF16, tag="ident")

        def dma(dst, src):
            nc.sync.dma_start(out=dst, in_=src)

        # startup chain for the first exp: QT-mb0 (xh+xl, wq blk0) then
        # KT-mb0; everything else ordered by first use (DMA is a single
        # serial queue in the cost model).
        # Preload the Exp activation table off the critical path: the first
        # real exp would otherwise pay the 1.28us table load at ~10us.
        dummy = misc.tile([128, 1], F32, tag="dummy")
        nc.vector.memset(dummy, 0.0)
        nc.scalar.activation(out=dummy, in_=dummy, func=Exp)

        dma(wqh_sb[:, 0], wqh[:, 0])
        dma(xh_sb[:, :, 0:512], xh[:, :, 0:512])
        dma(wql_sb[:, 0], wql[:, 0])
        dma(xl_sb[:, :, 0:512], xl[:, :, 0:512])
        dma(wkh_sb[:, 0], wkh[:, 0])
        dma(wkl_sb[:, 0], wkl[:, 0])
        dma(c_sb, consts[:, :])
        nc.vector.tensor_scalar_mul(maskb, c_sb[:, 8:16], NEG_INF)
        dma(xh_sb[:, :, 512:1024], xh[:, :, 512:1024])
        dma(xl_sb[:, :, 512:1024], xl[:, :, 512:1024])
        # blk1 weights: QK-mb1 filler halves start at h0c4
        for w_sb, w_d in ((wqh_sb, wqh), (wql_sb, wql),
                          (wkh_sb, wkh), (wkl_sb, wkl)):
            dma(w_sb[:, 1], w_d[:, 1])
        # wv first half (heads 0-3): V fillers start at h1c4
        dma(wvh_sb[:, 0], wvh[:, 0])
        dma(wvl_sb[:, 0], wvl[:, 0])
        dma(bv_bc, bv1[0:1, :].to_broadcast((128, GW)))
        dma(id_sb, ident[:, :])
        QT = p_qkt.tile([128, 4, S], BF16, tag="QT")
        KT = p_qkt.tile([128, 4, S], BF16, tag="KT")
        V_sb = p_v.tile([128, TC, HPG, HD + 1], BF16, tag="V")
        dma(V_sb[:, :, :, HD:HD + 1], ones.ap())
        for blk in range(2, 4):
            for w_sb, w_d in ((wqh_sb, wqh), (wql_sb, wql),
                              (wkh_sb, wkh), (wkl_sb, wkl)):
                dma(w_sb[:, blk], w_d[:, blk])
        # wv second half (heads 4-7): V-b fillers start at h4c3
        dma(wvh_sb[:, 1], wvh[:, 1])
        dma(wvl_sb[:, 1], wvl[:, 1])
        for blk in range(4):
            dma(woh_sb[:, blk, :], woh[:, blk, :])
            dma(wol_sb[:, blk, :], wol[:, blk, :])
        attn_sb = p_attn.tile([128, TC, GW], BF16, tag="attn")
        attnT_h = p_at.tile([128, 4, S], E4, tag="attnTh")
        attnT_l = p_at.tile([128, 4, S], E4, tag="attnTl")

        # ---- compute emitters ----
        pq_state = {}

        def proj_qk_half(dst, b_sb, wh_t, wl_t, mb, nh, nb):
            """Half (256 s-cols) of a 3-term fp8-DR projection block.
            nb=0 allocates the [128,512] psum tile; nb=1 emits the fused
            descale+bias DVE pass over the full 512."""
            key = (id(dst), mb, nh)
            if nb == 0:
                pq_state[key] = ps_proj.tile(
                    [128, 512], F32, tag="pproj", name=f"pq{mb}_{nh}")
            ps = pq_state[key]
            n0 = nh * 512
            ns = slice(n0 + nb * 256, n0 + (nb + 1) * 256)
            os_ = slice(nb * 256, (nb + 1) * 256)
            first = True
            for cp in range(4):
                cs = slice(2 * cp, 2 * cp + 2)
                for wt, xt in ((wh_t, xh_sb), (wl_t, xh_sb), (wh_t, xl_sb)):
                    last = (cp == 3 and xt is xl_sb)
                    nc.tensor.matmul(
                        ps[:, os_], wt[:, mb, cs, :], xt[:, cs, ns],
                        start=first, stop=last, perf_mode=DR,
                        skip_group_check=True)
                    first = False
            if nb == 1:
                nc.vector.tensor_scalar(
                    dst[:, mb, n0:n0 + 512], ps, DESCALE,
                    b_sb[:, mb:mb + 1], MULT, ADD)
                del pq_state[key]

        def proj_v_half(tb, nb):
            """V rows 128*tb for head-group nb (4 heads, 256 wv-cols);
            self-contained: 12 DR matmuls + fused descale+bias pass."""
            ps = ps_proj.tile([128, 512], F32, tag="pproj",
                              name=f"pv{tb}_{nb}")
            ts = slice(tb * 128, (tb + 1) * 128)
            first = True
            for cp in range(4):
                cs = slice(2 * cp, 2 * cp + 2)
                for wt, xt in ((wvh_sb, xh_sb), (wvl_sb, xh_sb),
                               (wvh_sb, xl_sb)):
                    last = (cp == 3 and xt is xl_sb)
                    nc.tensor.matmul(
                        ps[:, 0:256], xt[:, cs, ts], wt[:, nb, cs, :],
                        start=first, stop=last, perf_mode=DR,
                        skip_group_check=True)
                    first = False
            nc.vector.scalar_tensor_tensor(
                V_sb[:, tb, 4 * nb:4 * nb + 4, 0:HD],
                ps[:, 0:256].rearrange("p (h d) -> p h d", h=4), DESCALE,
                bv_bc[:, nb * 256:(nb + 1) * 256].rearrange(
                    "p (h d) -> p h d", h=4), MULT, ADD)

        def logits_exp(h, eT, tcn):
            mb, off = h // 2, (h % 2) * 64
            lg = ps_lg.tile([128, 1024], F32, tag="lg")
            for sh in range(2):
                nc.tensor.matmul(
                    lg[:, sh * 512:(sh + 1) * 512],
                    KT[off:off + 64, mb, tcn * 128:(tcn + 1) * 128],
                    QT[off:off + 64, mb, sh * 512:(sh + 1) * 512],
                    start=True, stop=True, tile_position=(off, 0))
            nc.scalar.activation(
                out=eT[:, tcn, :], in_=lg, func=Exp,
                bias=maskb[:, tcn:tcn + 1], scale=0.125)

        expT = {}
        av_state = {}

        def av_part(h, sbg, tcs):
            """Partial AV accumulation for head h, s-blocks 4*sbg.., over
            the t-chunks in `tcs` (split emission so av(7)'s last chunk is
            the only tail PE work)."""
            key = (h, sbg)
            if key not in av_state:
                av_state[key] = ps_po.tile(
                    [128, 4, HD + 1], F32, tag="po", name=f"po{h}_{sbg}")
            po = av_state[key]
            eT = expT[h]
            for i in range(4):
                sb = sbg * 4 + i
                for tcn in tcs:
                    nc.tensor.matmul(
                        po[:, i, :],
                        eT[:, tcn, sb * 128:(sb + 1) * 128],
                        V_sb[:, tcn, h, :],
                        start=(tcn == 0), stop=(tcn == TC - 1))

        def av_fin(h, sbg):
            po = av_state.pop((h, sbg))
            r4 = p_r4.tile([128, 4, 1], F32, tag="r4")
            nc.vector.reciprocal(r4, po[:, :, HD:HD + 1])
            nc.vector.tensor_mul(
                attn_sb[:, sbg * 4:(sbg + 1) * 4, h * HD:(h + 1) * HD],
                po[:, :, 0:HD], r4[:, :, 0:1].to_broadcast((128, 4, HD)))

        def av(h, sbg):
            av_part(h, sbg, range(TC))
            av_fin(h, sbg)

        def transpose_blk(blk):
            """attn s-block columns of gw-block blk -> attnT hi/lo fp8 rows
            (attn carries a x64 pow2 scale from the 1/64 ones column so the
            values sit in e4m3's normal range; descale folds into the
            output-copy pass). Borrows proj-pool psum slots via bitcast."""
            for sb in range(TC):
                bt = ps_proj.tile([128, 512], F32, tag="pproj",
                                  name=f"pt{blk}_{sb}")
                pt = bt[:, 0:128].bitcast(BF16)[:, 0:128]
                nc.tensor.matmul(
                    pt, attn_sb[:, sb, blk * 128:(blk + 1) * 128], id_sb,
                    start=True, stop=True, is_transpose=True)
                ss = slice(sb * 128, (sb + 1) * 128)
                nc.vector.tensor_copy(attnT_h[:, blk, ss], pt)
                nc.vector.tensor_sub(
                    attnT_l[:, blk, ss], pt, attnT_h[:, blk, ss])

        # ---------------- emission ----------------
        # QT-nh0, KT-nh0 first (they only need the first x halves), then
        # the nh1 halves as x's second half lands.
        for nb in range(2):
            proj_qk_half(QT, bq_sb, wqh_sb, wql_sb, 0, 0, nb)
        for nb in range(2):
            proj_qk_half(KT, bk_sb, wkh_sb, wkl_sb, 0, 0, nb)
        for nb in range(2):
            proj_qk_half(QT, bq_sb, wqh_sb, wql_sb, 0, 1, nb)
        for nb in range(2):
            proj_qk_half(KT, bk_sb, wkh_sb, wkl_sb, 0, 1, nb)

        def F_qk(dst_b, mb, nh, nb):
            dst, b_ = (QT, bq_sb) if dst_b == "q" else (KT, bk_sb)
            wh_t, wl_t = ((wqh_sb, wql_sb) if dst_b == "q"
                          else (wkh_sb, wkl_sb))
            return lambda: proj_qk_half(dst, b_, wh_t, wl_t, mb, nh, nb)

        def F_v(tb, nb):
            return lambda: proj_v_half(tb, nb)

        def F_av(h, g):
            return lambda: av(h, g)

        def F_avp(h, g, tcs):
            return lambda: av_part(h, g, tcs)

        def F_tp(blk):
            return lambda: transpose_blk(blk)

        # filler schedule: [h][tcn] -> list of emitters. Budget ~0.6us of
        # PE filler per 1.04us exp chunk; no fillers on the first chunks
        # (their inputs aren't DMA'd yet and PE head-of-line blocking
        # would starve ACT). Deadlines: QT/KT-mb(k) before head 2k's
        # logits, V head-group a (0-3) before av(0) at h2c5, group b
        # before av(4) at h6c4, av(h) done before head h+3 starts (expT
        # pool bufs=3), transpose blk b after av(2b+1), av(7) split so
        # only its last t-chunk trails the final exp.
        FILL = {
            (0, 4): [F_qk("q", 1, 0, 0)],
            (0, 5): [F_qk("q", 1, 0, 1)],
            (0, 6): [F_qk("q", 1, 1, 0)],
            (0, 7): [F_qk("q", 1, 1, 1)],
            (1, 0): [F_qk("k", 1, 0, 0)],
            (1, 1): [F_qk("k", 1, 0, 1)],
            (1, 2): [F_qk("k", 1, 1, 0)],
            (1, 3): [F_qk("k", 1, 1, 1)],
            (1, 4): [F_v(0, 0)], (1, 5): [F_v(1, 0)],
            (1, 6): [F_v(2, 0)], (1, 7): [F_v(3, 0)],
            (2, 0): [F_v(4, 0)], (2, 1): [F_v(5, 0)],
            (2, 2): [F_v(6, 0)], (2, 3): [F_v(7, 0)],
            (2, 4): [F_qk("q", 2, 0, 0)],
            (2, 5): [F_av(0, 0)],
            (2, 6): [F_qk("q", 2, 0, 1)],
            (2, 7): [F_av(0, 1)],
            (3, 0): [F_qk("q", 2, 1, 0)],
            (3, 1): [F_av(1, 0)],
            (3, 2): [F_qk("q", 2, 1, 1)],
            (3, 3): [F_qk("k", 2, 0, 0)],
            (3, 4): [F_qk("k", 2, 0, 1)],
            (3, 5): [F_av(1, 1)],
            (3, 6): [F_qk("k", 2, 1, 0)],
            (3, 7): [F_qk("k", 2, 1, 1)],
            (4, 0): [F_qk("q", 3, 0, 0)],
            (4, 1): [F_av(2, 0)],
            (4, 2): [F_qk("q", 3, 0, 1)],
            (4, 3): [F_v(0, 1)],
            (4, 4): [F_qk("q", 3, 1, 0)],
            (4, 5): [F_av(2, 1)],
            (4, 6): [F_qk("q", 3, 1, 1)],
            (4, 7): [F_v(1, 1)],
            (5, 0): [F_qk("k", 3, 0, 0)],
            (5, 1): [F_av(3, 0)],
            (5, 2): [F_qk("k", 3, 0, 1)],
            (5, 3): [F_qk("k", 3, 1, 0)],
            (5, 4): [F_qk("k", 3, 1, 1)],
            (5, 5): [F_av(3, 1)],
            (5, 6): [F_v(2, 1)],
            (5, 7): [F_v(3, 1)],
            (6, 0): [F_v(4, 1)],
            (6, 1): [F_v(5, 1)],
            (6, 2): [F_v(6, 1)],
            (6, 3): [F_v(7, 1)],
            (6, 4): [F_av(4, 0)],
            (6, 5): [F_tp(1)],
            (6, 6): [F_av(4, 1)],
            (6, 7): [F_tp(0)],
            (7, 0): [F_av(5, 0)],
            (7, 1): [F_av(5, 1)],
            (7, 2): [F_av(6, 0)],
            (7, 3): [F_av(6, 1)],
            (7, 4): [F_tp(2)],
        }

        for h in range(HPG):
            expT[h] = p_exp.tile([128, TC, S], BF16, tag="expT",
                                 name=f"expT{h}")
            for tcn in range(TC):
                for f in FILL.get((h, tcn), ()):
                    f()
                logits_exp(h, expT[h], tcn)

        lgps_cm.__exit__(None, None, None)
        opps_cm = tc.tile_pool(name="ps_op", bufs=2, space="PSUM")
        ps_op = opps_cm.__enter__()

        def transpose_one(blk, sb, use_po=False):
            # tail transposes: hi copy on ACT (idle after the last exp;
            # Copy shares the Exp table, no reload), lo subtract on DVE;
            # psum slots alternate between the proj and AV pools.
            if use_po:
                bt4 = ps_po.tile([128, 4, HD + 1], F32, tag="po",
                                 name=f"pt{blk}_{sb}")
                pt = bt4[:, 0, :].bitcast(BF16)[:, 0:128]
            else:
                bt = ps_proj.tile([128, 512], F32, tag="pproj",
                                  name=f"pt{blk}_{sb}")
                pt = bt[:, 0:128].bitcast(BF16)[:, 0:128]
            nc.tensor.matmul(
                pt, attn_sb[:, sb, blk * 128:(blk + 1) * 128], id_sb,
                start=True, stop=True, is_transpose=True)
            ss = slice(sb * 128, (sb + 1) * 128)
            nc.scalar.copy(attnT_h[:, blk, ss], pt)
            nc.vector.tensor_sub(
                attnT_l[:, blk, ss], pt, attnT_h[:, blk, ss])

        av(7, 0)
        av(7, 1)
        for sb in range(TC):
            transpose_one(3, sb, use_po=(sb % 2 == 1))

        ODESC = 1.0 / (64.0 * SW)   # attn x64 and wo x512 scales
        for st in range(TC):
            op = ps_op.tile([128, 1024], F32, tag="op")
            sts = slice(st * 128, (st + 1) * 128)
            for nh in range(2):
                first = True
                for nb in range(2):
                    os_ = slice(nh * 512 + nb * 256, nh * 512 + (nb + 1) * 256)
                    for p_ in range(2):
                        cs = slice(2 * p_, 2 * p_ + 2)
                        for at_t, wo_t in ((attnT_h, woh_sb),
                                           (attnT_l, woh_sb),
                                           (attnT_h, wol_sb)):
                            last = (nb == 1 and p_ == 1 and wo_t is wol_sb)
                            nc.tensor.matmul(
                                op[:, os_], at_t[:, cs, sts],
                                wo_t[:, cs, os_],
                                start=first, stop=last, perf_mode=DR,
                                skip_group_check=True)
                            first = False
            o_sb = p_o.tile([128, 1024], BF16, tag="o")
            nc.vector.tensor_scalar_mul(o_sb[:, 0:512], op[:, 0:512], ODESC)
            nc.scalar.mul(o_sb[:, 512:1024], op[:, 512:1024], ODESC)
            dma(out[st * 128:(st + 1) * 128, :], o_sb)

        for cm in (opps_cm, pops_cm, prps_cm, o_cm, at_cm, r4_cm, attn_cm,
                   exp_cm, v_cm, qkt_cm, w_cm, x_cm, misc_cm):
            cm.__exit__(None, None, None)

    nc.compile()
    return nc


_NC = {}


def _get_nc(nrep=1):
    if nrep not in _NC:
        _NC[nrep] = _build(nrep)
    return _NC[nrep]


E4NP = ml_dtypes.float8_e4m3


def _q8(a):
    """fp8 hi/lo split: a ~= hi + lo (both e4m3)."""
    hi = a.astype(E4NP)
    lo = (a - hi.astype(np.float32)).astype(E4NP)
    return hi, lo


def _chunk128(a):
    """[1024, M] -> [128, 8, M] partition-major chunking of the rows."""
    m = a.shape[1]
    return np.ascontiguousarray(a.reshape(HC, 128, m).transpose(1, 0, 2))


def kernel(x, mask, Wq, bq, Wk, bk, Wv, bv, Wo, bo, _trace=False):
    x = np.asarray(x, dtype=np.float32)
    mask = np.asarray(mask, dtype=np.float32)
    Wq, Wk, Wv, Wo = (np.asarray(w, dtype=np.float32) for w in (Wq, Wk, Wv, Wo))
    bq, bk, bv, bo = (np.asarray(b_, dtype=np.float32) for b_ in (bq, bk, bv, bo))

    nc = _get_nc()
    # ones column is 1/64: the softmax denominator comes out pre-scaled so
    # the normalized attn carries a x64 factor, putting it in e4m3's normal
    # range for the fp8 output projection (descaled by ODESC at the end).
    ones = np.full((128, TC, HPG, 1), 1.0 / 64.0, dtype=ml_dtypes.bfloat16)
    ident = np.eye(128, dtype=ml_dtypes.bfloat16)
    in_maps = []
    for c in range(NCORES):
        b, g = c // 2, c % 2
        sl = slice(g * GW, (g + 1) * GW)
        xh_, xl_ = _q8(np.ascontiguousarray(x[b].T) * SX)
        wq_h, wq_l = _q8(Wq[:, sl] * SW)
        wk_h, wk_l = _q8(Wk[:, sl] * SW)
        wv_h, wv_l = _q8(Wv[:, sl] * SW)
        wo_h, wo_l = _q8(np.ascontiguousarray(
            Wo[sl, :].reshape(4, 128, S).transpose(1, 0, 2)) * SW)

        def wblk(a, nblk):
            # [1024, 512] -> [128, nblk, HC, 512//nblk] (partition-major
            # rows, blk-major cols so per-blk DMA slices are contiguous)
            c = a.reshape(HC, 128, nblk, GW // nblk)
            return np.ascontiguousarray(c.transpose(1, 2, 0, 3))

        in_maps.append({
            "xh": _chunk128(xh_), "xl": _chunk128(xl_),
            "wqh": wblk(wq_h, 4), "wql": wblk(wq_l, 4),
            "wkh": wblk(wk_h, 4), "wkl": wblk(wk_l, 4),
            "wvh": wblk(wv_h, 2), "wvl": wblk(wv_l, 2),
            "woh": wo_h, "wol": wo_l,
            "consts": np.ascontiguousarray(np.concatenate([
                bq[sl].reshape(4, 128).T, bk[sl].reshape(4, 128).T,
                mask[b, 0, 0, :].reshape(8, 128).T], axis=1)),
            "bv1": np.ascontiguousarray(bv[sl]).reshape(1, GW),
            "ones": ones,
            "ident": ident,
        })
    # First execution after NEFF load can race engine table initialization.
    # Warm up, then run.
    run_bass_kernel_spmd(nc, in_maps, core_ids=list(range(NCORES)))
    res = run_bass_kernel_spmd(
        nc, in_maps, core_ids=list(range(NCORES)), trace=_trace)
    kernel.last_results = res
    parts = [res.results[c]["out"].astype(np.float32) for c in range(NCORES)]
    return np.stack(
        [parts[2 * b] + parts[2 * b + 1] + bo for b in range(B)]
    ).astype(np.float32)
